# revision 12
# baseline (speedup 1.0000x reference)
"""Trainium2 Bass kernel for the ACF (Attentive Collaborative Filtering) model.

Strategy (8 NeuronCores, positive-item axis P=512 sharded 64 items/core):
  - The only heavy compute is f_u_i_pos @ [Wi0_ix | Wc0_i]  ([3136,2048]@[2048,128]
    per core).  f is shipped per-core as fp8-e4m3, pre-transposed and packed
    row-tile-major on the host so every DMA is contiguous and the contraction
    dim (features) lands on SBUF partitions.  Matmuls run in fp8 DoubleRow mode
    (K=256 per pass).  Wcat is pre-scaled by 32 to dodge fp8 subnormals; the
    inverse scale is folded into the relu activation scale and the ginv
    broadcast constant (zero extra instructions).
  - One fused PE pass produces X^T = (f@Wi0_ix)^T on PSUM partitions 0-63 and
    H^T = (f@Wc0_i)^T on partitions 64-127.  Region softmax (over R=49) and the
    b-weighted sum over regions collapse to tiny per-core ops because
    all_x @ Wi0_ix == sum_r b*(f@Wi0_ix) -- all_x itself is never materialized.
  - Row tiles and epilogue tiles coincide (9 items per tile, and a 1-item final
    tile so the unavoidable serial dependency chain at the end operates on
    minimum-size tensors), so each tile's softmax/weighted-sum work pipelines
    behind the next tile's matmuls.
  - Item-level attention partials (sum of exp, exp-weighted p_pos sum) are
    DMA'd out per core; the cross-shard softmax combine (exact: no max shift is
    used anywhere, and softmax is shift-invariant so bc1/bi1 cancel) and the
    final tiny dot products happen on the host during unsharding.
"""

import numpy as np

import concourse.bacc as bacc
import concourse.tile as tile
from concourse import mybir
from concourse.bass_utils import run_bass_kernel_spmd

NCORES = 8
P_FULL = 512
R = 49
FEAT = 2048
D = 64
F = 200
P_LOC = P_FULL // NCORES      # 64 items per core
ROWS = P_LOC * R              # 3136 (item, region) rows per core
KC = FEAT // 128              # 16 contraction chunks of 128
RTS = [9 * R] * 7 + [R]       # row tiles (items*49): 7x441 + 1x49
OFFS = [sum(RTS[:i]) for i in range(len(RTS))]
WSCALE = 32.0                 # fp8 pre-scale on Wcat (folded back out downstream)

_CACHE: dict = {}


def _build():
    dt = mybir.dt
    AF = mybir.ActivationFunctionType
    DR = mybir.MatmulPerfMode.DoubleRow
    nc = bacc.Bacc()

    ft = nc.declare_dram_parameter("ft", [128, KC * ROWS], dt.float8e4, isOutput=False)
    wcat = nc.declare_dram_parameter("wcat", [128, KC * 128], dt.float8e4, isOutput=False)
    uc = nc.declare_dram_parameter("uc", [D, 1], dt.float32, isOutput=False)
    wc1 = nc.declare_dram_parameter("wc1", [D, 1], dt.float16, isOutput=False)
    et = nc.declare_dram_parameter("et", [D, P_LOC], dt.float32, isOutput=False)
    wi1 = nc.declare_dram_parameter("wi1", [D, 1], dt.float32, isOutput=False)
    ppos = nc.declare_dram_parameter("ppos", [P_LOC, F + 1], dt.float32, isOutput=False)
    out = nc.declare_dram_parameter("out", [1, F + 1], dt.float32, isOutput=True)

    with tile.TileContext(nc) as tc:
        with (
            tc.tile_pool(name="singles", bufs=1) as singles,
            tc.tile_pool(name="ftp", bufs=3) as ftp,
            tc.tile_pool(name="eep", bufs=3) as eep,
            tc.tile_pool(name="xap", bufs=3) as xap,
            tc.tile_pool(name="hrp", bufs=2) as hrp,
            tc.tile_pool(name="tmpp", bufs=2) as tmpp,
            tc.tile_pool(name="psbig", bufs=2, space="PSUM") as psbig,
            tc.tile_pool(name="pssmall", bufs=2, space="PSUM") as pssmall,
        ):
            # tile 0's first ft quarter, then wcat (both needed by matmul #1),
            # then the remaining quarters.
            ft0 = ftp.tile([128, KC, RTS[0]], dt.float8e4, tag="ft0")
            q = KC // 4
            nc.sync.dma_start(out=ft0[:, 0:q], in_=ft[:, 0 : q * RTS[0]])
            wcat_sb = singles.tile([128, KC, 128], dt.float8e4)
            nc.sync.dma_start(out=wcat_sb, in_=wcat.rearrange("p (k m) -> p k m", k=KC))
            for qi in range(1, 4):
                nc.sync.dma_start(
                    out=ft0[:, qi * q : (qi + 1) * q],
                    in_=ft[:, qi * q * RTS[0] : (qi + 1) * q * RTS[0]],
                )

            # small constants on the gpsimd (SWDGE) queue so they don't delay ft
            ucw_sb = singles.tile([128, 1], dt.float32)
            nc.gpsimd.dma_start(out=ucw_sb[D : 2 * D, :], in_=uc[:])
            wc1_sb = singles.tile([128, 1], dt.float16)
            nc.gpsimd.dma_start(out=wc1_sb[D : 2 * D, :], in_=wc1[:])
            et_sb = singles.tile([D, P_LOC], dt.float32)
            nc.gpsimd.dma_start(out=et_sb, in_=et[:])
            wi1_sb = singles.tile([D, 1], dt.float32)
            nc.gpsimd.dma_start(out=wi1_sb, in_=wi1[:])
            ppos_sb = singles.tile([P_LOC, F + 1], dt.float32)
            nc.gpsimd.dma_start(out=ppos_sb, in_=ppos[:])

            ones16_sb = singles.tile([1, P_LOC], dt.float16)
            nc.vector.memset(ones16_sb, 1.0)
            ones32_sb = singles.tile([1, P_LOC], dt.float32)
            nc.vector.memset(ones32_sb, 1.0 / WSCALE)

            at_sb = singles.tile([P_LOC, P_LOC], dt.float32)
            g_sb = singles.tile([1, P_LOC], dt.float32)

            # ft DMAs are merged in pairs of row tiles (bigger transfers keep
            # the 16 SDMA engines at line rate); processing stays per-tile.
            DMA_GROUPS = [[0], [1, 2], [3, 4], [5, 6], [7]]
            for grp in DMA_GROUPS:
                if grp == [0]:
                    ftg = ft0
                else:
                    w = sum(RTS[t] for t in grp)
                    ftg = ftp.tile([128, KC, w], dt.float8e4, tag="ftg")
                    nc.sync.dma_start(
                        out=ftg,
                        in_=ft[:, KC * OFFS[grp[0]] : KC * (OFFS[grp[0]] + w)],
                    )
                for ti, t in enumerate(grp):
                    rt = RTS[t]
                    lo = OFFS[t] - OFFS[grp[0]]
                    ipt = rt // R
                    io = OFFS[t] // R
                    isl = slice(io, io + ipt)
                    ps = psbig.tile([128, rt], dt.float32, tag="ps")
                    for g in range(KC // 2):
                        nc.tensor.matmul(
                            ps,
                            wcat_sb[:, 2 * g : 2 * g + 2],
                            ftg[:, 2 * g : 2 * g + 2, lo : lo + rt],
                            start=(g == 0),
                            stop=(g == KC // 2 - 1),
                            perf_mode=DR,
                        )
                    # H^T slice (partitions 64-127): relu(H/WSCALE + uc), f16
                    hr = hrp.tile([128, rt], dt.float16, tag="hr")
                    nc.scalar.activation(
                        hr[D : 2 * D], ps[D : 2 * D], AF.Relu,
                        bias=ucw_sb[D : 2 * D, 0:1], scale=1.0 / WSCALE,
                    )
                    # region logits = Wc1^T @ Hr  (row group 64..127), then exp
                    psl = pssmall.tile([1, rt], dt.float32, tag="small")
                    nc.tensor.matmul(psl, wc1_sb[D : 2 * D, :], hr[D : 2 * D])
                    e_t = eep.tile([1, rt], dt.float16, tag="et")
                    nc.scalar.activation(e_t, psl, AF.Exp)

                    # X^T slice (partitions 0-63) -> SBUF (exact f32 copy;
                    # the scalar engine's Copy path is low-precision)
                    xa_t = xap.tile([P_LOC, rt], dt.float32, tag="xat")
                    nc.vector.tensor_copy(xa_t, ps[0:D])

                    # epilogue for this tile's items:
                    #   at[:, p] = sum_r e[p,r] * X^T[:, (p,r)],  g[p] = sum_r e
                    pse = pssmall.tile([P_LOC, rt], dt.float32, tag="small")
                    nc.tensor.matmul(pse, ones16_sb, e_t)
                    tmp = tmpp.tile([P_LOC, rt], dt.float32, tag="tmp")
                    nc.vector.tensor_mul(tmp, xa_t, pse)
                    nc.vector.tensor_reduce(
                        out=at_sb[:, isl],
                        in_=tmp.rearrange("p (i r) -> p i r", r=R),
                        axis=mybir.AxisListType.X,
                        op=mybir.AluOpType.add,
                    )
                    nc.vector.tensor_reduce(
                        out=g_sb[0:1, isl],
                        in_=e_t.rearrange("a (i r) -> a i r", r=R),
                        axis=mybir.AxisListType.X,
                        op=mybir.AluOpType.add,
                    )

            # S^T = relu(E^T + A^T_unnorm / (G * WSCALE))
            ginv_sb = singles.tile([1, P_LOC], dt.float32)
            nc.vector.reciprocal(ginv_sb, g_sb)
            psg = pssmall.tile([D, P_LOC], dt.float32, tag="small")
            nc.tensor.matmul(psg, ones32_sb[0:1, 0:D], ginv_sb)
            st_sb = singles.tile([D, P_LOC], dt.float32)
            nc.vector.tensor_mul(st_sb, at_sb[0:D], psg)
            nc.vector.tensor_add(st_sb, st_sb, et_sb)
            nc.scalar.activation(st_sb, st_sb, AF.Relu)

            # item logits, exp, and the exp-weighted [p_pos | 1] sum
            psa = pssmall.tile([P_LOC, 1], dt.float32, tag="small")
            nc.tensor.matmul(psa, st_sb, wi1_sb)
            ecol_sb = singles.tile([P_LOC, 1], dt.float32)
            nc.scalar.activation(ecol_sb, psa, AF.Exp)
            psv = pssmall.tile([1, F + 1], dt.float32, tag="small")
            nc.tensor.matmul(psv, ecol_sb, ppos_sb)
            out_sb = singles.tile([1, F + 1], dt.float32)
            nc.vector.tensor_copy(out_sb, psv)
            nc.sync.dma_start(out=out[:], in_=out_sb)

    nc.finalize()
    return nc


def _get_nc():
    if "nc" not in _CACHE:
        _CACHE["nc"] = _build()
    return _CACHE["nc"]


def kernel(**inputs) -> np.ndarray:
    f = np.asarray(inputs["f_u_i_pos"], dtype=np.float32)
    Gu = np.asarray(inputs["Gu"], np.float32)
    Gi = np.asarray(inputs["Gi"], np.float32)
    Pi = np.asarray(inputs["Pi"], np.float32)
    user = int(np.asarray(inputs["user"]))
    item = int(np.asarray(inputs["item"]))
    pos = np.asarray(inputs["user_pos"]).reshape(-1).astype(np.int64)

    g_u = Gu[user]
    gamma_i = Gi[item]
    p_i_item = Pi[item]
    g_pos = Gi[pos]
    p_pos = Pi[pos]

    uc = g_u @ np.asarray(inputs["Wc0_u"], np.float32) + np.asarray(inputs["bc0"], np.float32)
    ui = g_u @ np.asarray(inputs["Wi0_u"], np.float32) + np.asarray(inputs["bi0"], np.float32)
    E = (
        ui[None]
        + g_pos @ np.asarray(inputs["Wi0_iv"], np.float32)
        + p_pos @ np.asarray(inputs["Wi0_ip"], np.float32)
    )  # [512, 64]
    wc1 = np.asarray(inputs["Wc1"], np.float32).reshape(D, 1)
    wi1 = np.asarray(inputs["Wi1"], np.float32).reshape(D, 1)
    Wcat = np.concatenate(
        [np.asarray(inputs["Wi0_ix"], np.float32), np.asarray(inputs["Wc0_i"], np.float32)],
        axis=1,
    )  # [2048, 128]

    # Host packing: SBUF-layout-exact, so every device DMA is contiguous.
    import ml_dtypes

    f8 = ml_dtypes.float8_e4m3
    wcat_host = np.ascontiguousarray(
        (Wcat * WSCALE).reshape(KC, 128, 128).transpose(1, 0, 2).reshape(128, KC * 128)
    ).astype(f8)

    f16_rows = f[0].reshape(P_FULL * R, FEAT).astype(f8)  # [25088, 2048]

    in_maps = []
    for c in range(NCORES):
        fc_t = f16_rows[c * ROWS : (c + 1) * ROWS].T  # [2048, 3136] (view)
        a = fc_t.reshape(KC, 128, ROWS)
        ft_host = np.concatenate(
            [
                a[:, :, OFFS[t] : OFFS[t] + rt].transpose(1, 0, 2).reshape(128, KC * rt)
                for t, rt in enumerate(RTS)
            ],
            axis=1,
        )  # [128, KC*ROWS], row-tile-major, contiguous per partition slice
        ppos_c = np.concatenate(
            [p_pos[c * P_LOC : (c + 1) * P_LOC], np.ones((P_LOC, 1), np.float32)], axis=1
        ).astype(np.float32)
        in_maps.append(
            {
                "ft": np.ascontiguousarray(ft_host),
                "wcat": wcat_host,
                "uc": uc.reshape(D, 1).astype(np.float32),
                "wc1": wc1.astype(np.float16),
                "et": np.ascontiguousarray(E[c * P_LOC : (c + 1) * P_LOC].T).astype(
                    np.float32
                ),
                "wi1": wi1,
                "ppos": ppos_c,
            }
        )

    nc = _get_nc()
    _CACHE["in_maps"] = in_maps
    res = run_bass_kernel_spmd(nc, in_maps, core_ids=list(range(NCORES)))
    outs = [np.asarray(res.results[c]["out"][0], np.float64) for c in range(NCORES)]

    V = sum(o[:F] for o in outs)
    S = sum(float(o[F]) for o in outs)
    all_a = V / S
    xui = np.float32(np.dot(g_u.astype(np.float64) + all_a, gamma_i.astype(np.float64)))
    return (np.array(xui, np.float32), g_u, gamma_i, p_i_item)


# revision 13
# speedup vs baseline: 1.1422x; 1.1422x over previous
"""Trainium2 Bass kernel for the ACF (Attentive Collaborative Filtering) model.

Strategy (8 NeuronCores, positive-item axis P=512 sharded 64 items/core):
  - The only heavy compute is f_u_i_pos @ [Wi0_ix | Wc0_i]  ([3136,2048]@[2048,128]
    per core).  f is shipped per-core as fp8-e4m3, pre-transposed and packed
    row-tile-major on the host so every DMA is contiguous and the contraction
    dim (features) lands on SBUF partitions.  Matmuls run in fp8 DoubleRow mode
    (K=256 per pass).  Wcat is pre-scaled by 32 to dodge fp8 subnormals; the
    inverse scale is folded into the relu activation scale and the ginv
    broadcast constant (zero extra instructions).
  - One fused PE pass produces X^T = (f@Wi0_ix)^T on PSUM partitions 0-63 and
    H^T = (f@Wc0_i)^T on partitions 64-127.  Region softmax (over R=49) and the
    b-weighted sum over regions collapse to tiny per-core ops because
    all_x @ Wi0_ix == sum_r b*(f@Wi0_ix) -- all_x itself is never materialized.
  - Row tiles and epilogue tiles coincide (9 items per tile, and a 1-item final
    tile so the unavoidable serial dependency chain at the end operates on
    minimum-size tensors), so each tile's softmax/weighted-sum work pipelines
    behind the next tile's matmuls.
  - Item-level attention partials (sum of exp, exp-weighted p_pos sum) are
    DMA'd out per core; the cross-shard softmax combine (exact: no max shift is
    used anywhere, and softmax is shift-invariant so bc1/bi1 cancel) and the
    final tiny dot products happen on the host during unsharding.
"""

import numpy as np

import concourse.bacc as bacc
import concourse.tile as tile
from concourse import mybir
from concourse.bass_utils import run_bass_kernel_spmd

NCORES = 8
P_FULL = 512
R = 49
FEAT = 2048
D = 64
F = 200
P_LOC = P_FULL // NCORES      # 64 items per core
ROWS = P_LOC * R              # 3136 (item, region) rows per core
KC = FEAT // 128              # 16 contraction chunks of 128
RTS = [9 * R] * 7 + [R]       # row tiles (items*49): 7x441 + 1x49
OFFS = [sum(RTS[:i]) for i in range(len(RTS))]
DMA_RTS = [441, 882, 882, 882, 49]   # host packing granularity = DMA groups
DMA_OFFS = [sum(DMA_RTS[:i]) for i in range(len(DMA_RTS))]
WSCALE = 32.0                 # fp8 pre-scale on Wcat (folded back out downstream)

_CACHE: dict = {}


def _build():
    dt = mybir.dt
    AF = mybir.ActivationFunctionType
    DR = mybir.MatmulPerfMode.DoubleRow
    nc = bacc.Bacc()

    ft = nc.declare_dram_parameter("ft", [128, KC * ROWS], dt.float8e4, isOutput=False)
    wcat = nc.declare_dram_parameter("wcat", [128, KC * 128], dt.float8e4, isOutput=False)
    uc = nc.declare_dram_parameter("uc", [D, 1], dt.float32, isOutput=False)
    wc1 = nc.declare_dram_parameter("wc1", [D, 1], dt.float16, isOutput=False)
    et = nc.declare_dram_parameter("et", [D, P_LOC], dt.float32, isOutput=False)
    wi1 = nc.declare_dram_parameter("wi1", [D, 1], dt.float32, isOutput=False)
    ppos = nc.declare_dram_parameter("ppos", [P_LOC, F + 1], dt.float32, isOutput=False)
    out = nc.declare_dram_parameter("out", [1, F + 1], dt.float32, isOutput=True)

    with tile.TileContext(nc) as tc:
        with (
            tc.tile_pool(name="singles", bufs=1) as singles,
            tc.tile_pool(name="ftp", bufs=3) as ftp,
            tc.tile_pool(name="eep", bufs=3) as eep,
            tc.tile_pool(name="xap", bufs=3) as xap,
            tc.tile_pool(name="hrp", bufs=2) as hrp,
            tc.tile_pool(name="tmpp", bufs=2) as tmpp,
            tc.tile_pool(name="psbig", bufs=2, space="PSUM") as psbig,
            tc.tile_pool(name="pssmall", bufs=2, space="PSUM") as pssmall,
        ):
            # tile 0's first ft quarter, then wcat (both needed by matmul #1),
            # then the remaining quarters.
            ft0 = ftp.tile([128, KC, RTS[0]], dt.float8e4, tag="ft0")
            q = KC // 4
            nc.sync.dma_start(out=ft0[:, 0:q], in_=ft[:, 0 : q * RTS[0]])
            wcat_sb = singles.tile([128, KC, 128], dt.float8e4)
            nc.sync.dma_start(out=wcat_sb, in_=wcat.rearrange("p (k m) -> p k m", k=KC))
            for qi in range(1, 4):
                nc.sync.dma_start(
                    out=ft0[:, qi * q : (qi + 1) * q],
                    in_=ft[:, qi * q * RTS[0] : (qi + 1) * q * RTS[0]],
                )

            # small constants on the gpsimd (SWDGE) queue so they don't delay ft
            ucw_sb = singles.tile([128, 1], dt.float32)
            nc.gpsimd.dma_start(out=ucw_sb[D : 2 * D, :], in_=uc[:])
            wc1_sb = singles.tile([128, 1], dt.float16)
            nc.gpsimd.dma_start(out=wc1_sb[D : 2 * D, :], in_=wc1[:])
            et_sb = singles.tile([D, P_LOC], dt.float32)
            nc.gpsimd.dma_start(out=et_sb, in_=et[:])
            wi1_sb = singles.tile([D, 1], dt.float32)
            nc.gpsimd.dma_start(out=wi1_sb, in_=wi1[:])
            ppos_sb = singles.tile([P_LOC, F + 1], dt.float32)
            nc.gpsimd.dma_start(out=ppos_sb, in_=ppos[:])

            ones16_sb = singles.tile([1, P_LOC], dt.float16)
            nc.vector.memset(ones16_sb, 1.0)
            ones32_sb = singles.tile([1, P_LOC], dt.float32)
            nc.vector.memset(ones32_sb, 1.0 / WSCALE)

            at_sb = singles.tile([P_LOC, P_LOC], dt.float32)
            g_sb = singles.tile([1, P_LOC], dt.float32)

            # ft DMAs are merged in pairs of row tiles (bigger transfers keep
            # the 16 SDMA engines at line rate); processing stays per-tile.
            DMA_GROUPS = [[0], [1, 2], [3, 4], [5, 6], [7]]
            for grp in DMA_GROUPS:
                if grp == [0]:
                    ftg = ft0
                else:
                    w = sum(RTS[t] for t in grp)
                    ftg = ftp.tile([128, KC, w], dt.float8e4, tag="ftg")
                    nc.sync.dma_start(
                        out=ftg,
                        in_=ft[:, KC * OFFS[grp[0]] : KC * (OFFS[grp[0]] + w)],
                    )
                for ti, t in enumerate(grp):
                    rt = RTS[t]
                    lo = OFFS[t] - OFFS[grp[0]]
                    ipt = rt // R
                    io = OFFS[t] // R
                    isl = slice(io, io + ipt)
                    ps = psbig.tile([128, rt], dt.float32, tag="ps")
                    for g in range(KC // 2):
                        nc.tensor.matmul(
                            ps,
                            wcat_sb[:, 2 * g : 2 * g + 2],
                            ftg[:, 2 * g : 2 * g + 2, lo : lo + rt],
                            start=(g == 0),
                            stop=(g == KC // 2 - 1),
                            perf_mode=DR,
                        )
                    # H^T slice (partitions 64-127): relu(H/WSCALE + uc), f16
                    hr = hrp.tile([128, rt], dt.float16, tag="hr")
                    nc.scalar.activation(
                        hr[D : 2 * D], ps[D : 2 * D], AF.Relu,
                        bias=ucw_sb[D : 2 * D, 0:1], scale=1.0 / WSCALE,
                    )
                    # region logits = Wc1^T @ Hr  (row group 64..127), then exp
                    psl = pssmall.tile([1, rt], dt.float32, tag="small")
                    nc.tensor.matmul(psl, wc1_sb[D : 2 * D, :], hr[D : 2 * D])
                    e_t = eep.tile([1, rt], dt.float16, tag="et")
                    nc.scalar.activation(e_t, psl, AF.Exp)

                    # X^T slice (partitions 0-63) -> SBUF (exact f32 copy;
                    # the scalar engine's Copy path is low-precision)
                    xa_t = xap.tile([P_LOC, rt], dt.float32, tag="xat")
                    nc.vector.tensor_copy(xa_t, ps[0:D])

                    # epilogue for this tile's items:
                    #   at[:, p] = sum_r e[p,r] * X^T[:, (p,r)],  g[p] = sum_r e
                    pse = pssmall.tile([P_LOC, rt], dt.float32, tag="small")
                    nc.tensor.matmul(pse, ones16_sb, e_t)
                    tmp = tmpp.tile([P_LOC, rt], dt.float32, tag="tmp")
                    nc.vector.tensor_mul(tmp, xa_t, pse)
                    nc.vector.tensor_reduce(
                        out=at_sb[:, isl],
                        in_=tmp.rearrange("p (i r) -> p i r", r=R),
                        axis=mybir.AxisListType.X,
                        op=mybir.AluOpType.add,
                    )
                    nc.vector.tensor_reduce(
                        out=g_sb[0:1, isl],
                        in_=e_t.rearrange("a (i r) -> a i r", r=R),
                        axis=mybir.AxisListType.X,
                        op=mybir.AluOpType.add,
                    )

            # S^T = relu(E^T + A^T_unnorm / (G * WSCALE))
            ginv_sb = singles.tile([1, P_LOC], dt.float32)
            nc.vector.reciprocal(ginv_sb, g_sb)
            psg = pssmall.tile([D, P_LOC], dt.float32, tag="small")
            nc.tensor.matmul(psg, ones32_sb[0:1, 0:D], ginv_sb)
            st_sb = singles.tile([D, P_LOC], dt.float32)
            nc.vector.tensor_mul(st_sb, at_sb[0:D], psg)
            nc.vector.tensor_add(st_sb, st_sb, et_sb)
            nc.scalar.activation(st_sb, st_sb, AF.Relu)

            # item logits, exp, and the exp-weighted [p_pos | 1] sum
            psa = pssmall.tile([P_LOC, 1], dt.float32, tag="small")
            nc.tensor.matmul(psa, st_sb, wi1_sb)
            ecol_sb = singles.tile([P_LOC, 1], dt.float32)
            nc.scalar.activation(ecol_sb, psa, AF.Exp)
            psv = pssmall.tile([1, F + 1], dt.float32, tag="small")
            nc.tensor.matmul(psv, ecol_sb, ppos_sb)
            out_sb = singles.tile([1, F + 1], dt.float32)
            nc.vector.tensor_copy(out_sb, psv)
            nc.sync.dma_start(out=out[:], in_=out_sb)

    nc.finalize()
    return nc


def _get_nc():
    if "nc" not in _CACHE:
        _CACHE["nc"] = _build()
    return _CACHE["nc"]


def kernel(**inputs) -> np.ndarray:
    f = np.asarray(inputs["f_u_i_pos"], dtype=np.float32)
    Gu = np.asarray(inputs["Gu"], np.float32)
    Gi = np.asarray(inputs["Gi"], np.float32)
    Pi = np.asarray(inputs["Pi"], np.float32)
    user = int(np.asarray(inputs["user"]))
    item = int(np.asarray(inputs["item"]))
    pos = np.asarray(inputs["user_pos"]).reshape(-1).astype(np.int64)

    g_u = Gu[user]
    gamma_i = Gi[item]
    p_i_item = Pi[item]
    g_pos = Gi[pos]
    p_pos = Pi[pos]

    uc = g_u @ np.asarray(inputs["Wc0_u"], np.float32) + np.asarray(inputs["bc0"], np.float32)
    ui = g_u @ np.asarray(inputs["Wi0_u"], np.float32) + np.asarray(inputs["bi0"], np.float32)
    E = (
        ui[None]
        + g_pos @ np.asarray(inputs["Wi0_iv"], np.float32)
        + p_pos @ np.asarray(inputs["Wi0_ip"], np.float32)
    )  # [512, 64]
    wc1 = np.asarray(inputs["Wc1"], np.float32).reshape(D, 1)
    wi1 = np.asarray(inputs["Wi1"], np.float32).reshape(D, 1)
    Wcat = np.concatenate(
        [np.asarray(inputs["Wi0_ix"], np.float32), np.asarray(inputs["Wc0_i"], np.float32)],
        axis=1,
    )  # [2048, 128]

    # Host packing: SBUF-layout-exact, so every device DMA is contiguous.
    import ml_dtypes

    f8 = ml_dtypes.float8_e4m3
    wcat_host = np.ascontiguousarray(
        (Wcat * WSCALE).reshape(KC, 128, 128).transpose(1, 0, 2).reshape(128, KC * 128)
    ).astype(f8)

    f16_rows = f[0].reshape(P_FULL * R, FEAT).astype(f8)  # [25088, 2048]

    in_maps = []
    for c in range(NCORES):
        fc_t = f16_rows[c * ROWS : (c + 1) * ROWS].T  # [2048, 3136] (view)
        a = fc_t.reshape(KC, 128, ROWS)
        ft_host = np.concatenate(
            [
                a[:, :, o : o + rt].transpose(1, 0, 2).reshape(128, KC * rt)
                for o, rt in zip(DMA_OFFS, DMA_RTS)
            ],
            axis=1,
        )  # [128, KC*ROWS], DMA-group-major, contiguous per partition slice
        ppos_c = np.concatenate(
            [p_pos[c * P_LOC : (c + 1) * P_LOC], np.ones((P_LOC, 1), np.float32)], axis=1
        ).astype(np.float32)
        in_maps.append(
            {
                "ft": np.ascontiguousarray(ft_host),
                "wcat": wcat_host,
                "uc": uc.reshape(D, 1).astype(np.float32),
                "wc1": wc1.astype(np.float16),
                "et": np.ascontiguousarray(E[c * P_LOC : (c + 1) * P_LOC].T).astype(
                    np.float32
                ),
                "wi1": wi1,
                "ppos": ppos_c,
            }
        )

    nc = _get_nc()
    _CACHE["in_maps"] = in_maps
    res = run_bass_kernel_spmd(nc, in_maps, core_ids=list(range(NCORES)))
    outs = [np.asarray(res.results[c]["out"][0], np.float64) for c in range(NCORES)]
    _CACHE["last_outs"] = outs

    V = sum(o[:F] for o in outs)
    S = sum(float(o[F]) for o in outs)
    all_a = V / S
    xui = np.float32(np.dot(g_u.astype(np.float64) + all_a, gamma_i.astype(np.float64)))
    return (np.array(xui, np.float32), g_u, gamma_i, p_i_item)
